# revision 10
# baseline (speedup 1.0000x reference)
"""TRN2 Bass kernel for nn_CPPScatterOpModule (gather -> products -> scatter-add).

Math (per feature f, row r, with shared channel-index lists idx0/1/2 of len N):
    g_k = x[idx_k]                                   (gather along C)
    part0[c] += mp3 via each idx_k   where mp3 = g0*g1*g2
    part1[c] += g1*g2 via idx0, g0*g2 via idx1, g0*g1 via idx2
    out = concat(part0, part1)                       [2F, R, C]

Strategy: R is sharded 8 ways (data-parallel, no comms). Per core the tensor
is laid out channel-major: X_T [C, RS*F] fp32, so a gather/scatter of one
channel is a contiguous 2KB row -> MoE-style dma_gather / dma_scatter_add.

dma_scatter_add's destination-side accumulate is NOT atomic between DMA
engines, so duplicate targets inside one instruction lose updates. Indices
are known at kernel-call time, so we schedule the N tokens into rounds such
that within a round each index list has unique values; rounds targeting the
same output buffer serialize via the Tile dependency tracker (verified
exact on HW), while the two output chains and the gathers run concurrently.
The round permutation is folded into the gather lists, so it is free.
"""

import os
import sys

for _p in ("/opt/trn_rl_repo", "/root/.axon_site/_ro/trn_rl_repo"):
    if os.path.isdir(_p) and _p not in sys.path:
        sys.path.append(_p)

import numpy as np

F_IN = 4
R = 1024
C = 4096
N = 8192
NCORES = 8
RS = R // NCORES  # rows per core
E = F_IN * RS  # fp32 elements per channel row per core (2048B)
CAP = int(os.environ.get("BASS_CAP", "768"))  # tokens per round (ring-safe; <=1024 for 2KB rows)
SLOTS = CAP // 128  # token slots in partition-major tile


def _schedule_rounds(idx_lists):
    """Assign tokens 0..N-1 to rounds of <=CAP slots such that inside a round
    no index list repeats a value. Greedy, least-filled-first. Returns
    (n_rounds, rounds) with rounds = list of token-id lists."""
    n = len(idx_lists[0])
    rounds = []  # (fill list, [set per idx list])
    order = list(range(n))
    for t in order:
        vals = [int(l[t]) for l in idx_lists]
        placed = False
        # try rounds in ascending fill so rounds stay balanced
        for ri in sorted(range(len(rounds)), key=lambda i: len(rounds[i][0])):
            toks, sets = rounds[ri]
            if len(toks) >= CAP:
                continue
            if any(v in s for v, s in zip(vals, sets)):
                continue
            toks.append(t)
            for v, s in zip(vals, sets):
                s.add(v)
            placed = True
            break
        if not placed:
            rounds.append(([t], [{v} for v in vals]))
    return len(rounds), [r[0] for r in rounds]


def _wrap16(arr2d):
    """[NR, CAP] int -> [128, NR*CAP//16] int16 wrapped (i at [i%16, i//16])
    and replicated across the 8 gpsimd partition groups."""
    nr = arr2d.shape[0]
    w = arr2d.astype(np.int16).reshape(nr, CAP // 16, 16)  # [NR, slot, lane]
    w = w.transpose(2, 0, 1).reshape(16, nr * (CAP // 16))  # [16, NR*CAP/16]
    return np.ascontiguousarray(np.tile(w, (8, 1)))


def _build_index_tiles(idx0, idx1, idx2):
    idx_lists = [np.asarray(idx0), np.asarray(idx1), np.asarray(idx2)]
    nr, rounds = _schedule_rounds(idx_lists)
    fills = []
    g_tiles = np.zeros((3, nr, CAP), np.int64)  # gather: pad with 0 (valid row)
    s_tiles = np.full((3, nr, CAP), -1, np.int64)  # scatter: pad with -1 (skip)
    for ri, toks in enumerate(rounds):
        fills.append(len(toks))
        for k in range(3):
            v = idx_lists[k][toks]
            g_tiles[k, ri, : len(toks)] = v
            s_tiles[k, ri, : len(toks)] = v
    g_wrapped = [_wrap16(g_tiles[k]) for k in range(3)]
    s_wrapped = [_wrap16(s_tiles[k]) for k in range(3)]
    return nr, fills, g_wrapped, s_wrapped


def _build_nc(nr, fills):
    import concourse.bacc as bacc
    import concourse.tile as tile
    from concourse import mybir

    W = CAP // 16  # idx columns per round

    nc = bacc.Bacc(
        "TRN2", target_bir_lowering=False, debug=False, num_swdge_queues=4
    )
    xt = nc.dram_tensor("xt", [C, E], mybir.dt.float32, kind="ExternalInput")
    gl = [
        nc.dram_tensor(f"gl{k}", [128, nr * W], mybir.dt.int16, kind="ExternalInput")
        for k in range(3)
    ]
    sl = [
        nc.dram_tensor(f"sl{k}", [128, nr * W], mybir.dt.int16, kind="ExternalInput")
        for k in range(3)
    ]
    out0 = nc.dram_tensor("out0", [C, E], mybir.dt.float32, kind="ExternalOutput")
    out1 = nc.dram_tensor("out1", [C, E], mybir.dt.float32, kind="ExternalOutput")
    rot = int(os.environ.get("BASS_ROT", "1"))
    out0r = [out0] + [nc.dram_tensor(f"out0r{i}", [C, E], mybir.dt.float32)
                      for i in range(1, rot)]
    out1r = [out1] + [nc.dram_tensor(f"out1r{i}", [C, E], mybir.dt.float32)
                      for i in range(1, rot)]

    f32 = mybir.dt.float32
    repeat = int(os.environ.get("BASS_KERNEL_REPEAT", "1"))
    single_packet = os.environ.get("BASS_SP", "1") != "0"
    gq = [int(q) for q in os.environ.get("BASS_GQ", "0").split(",")]
    gbufs = int(os.environ.get("BASS_GBUFS", "2"))
    pbufs = int(os.environ.get("BASS_PBUFS", "2"))
    skip = set(os.environ.get("BASS_SKIP", "").split(","))
    with tile.TileContext(nc) as tc:
        with (
            tc.tile_pool(name="idx", bufs=1) as ipool,
            tc.tile_pool(name="work", bufs=2) as wpool,
        ):
            gl_t = [ipool.tile([128, nr * W], mybir.dt.int16, name=f"glt{k}", tag=f"gl{k}") for k in range(3)]
            sl_t = [ipool.tile([128, nr * W], mybir.dt.int16, name=f"slt{k}", tag=f"sl{k}") for k in range(3)]
            for k in range(3):
                nc.sync.dma_start(out=gl_t[k][:], in_=gl[k][:])
                nc.sync.dma_start(out=sl_t[k][:], in_=sl[k][:])

            # zero both outputs (scatter-add accumulates in DRAM)
            z = ipool.tile([128, E], f32)
            nc.gpsimd.memset(z[:], 0.0)
            for r in range(0, C, 128):
                for b0, b1 in zip(out0r, out1r):
                    nc.sync.dma_start(out=b0[r : r + 128, :], in_=z[:])
                    nc.sync.dma_start(out=b1[r : r + 128, :], in_=z[:])

            for rep in range(repeat):
              for ri in range(nr):
                iw = slice(ri * W, (ri + 1) * W)
                g = [wpool.tile([128, SLOTS, E], f32, name=f"g{k}_{rep}_{ri}", tag=f"g{k}", bufs=gbufs) for k in range(3)]
                for k in range(3):
                    if "gather" in skip:
                        break
                    nc.gpsimd.dma_gather(
                        out_ap=g[k][:],
                        in_ap=xt[:],
                        idxs_ap=gl_t[k][:, iw],
                        num_idxs=CAP,
                        num_idxs_reg=CAP,
                        elem_size=E,
                        queue_num=gq[(ri * 3 + k) % len(gq)],
                        single_packet=single_packet,
                    )
                t12 = wpool.tile([128, SLOTS, E], f32, tag="t12", bufs=pbufs)
                t02 = wpool.tile([128, SLOTS, E], f32, tag="t02", bufs=pbufs)
                t01 = wpool.tile([128, SLOTS, E], f32, tag="t01", bufs=pbufs)
                mp3 = wpool.tile([128, SLOTS, E], f32, tag="mp3", bufs=pbufs)
                if "mul" not in skip:
                    nc.vector.tensor_mul(t12[:], g[1][:], g[2][:])
                    nc.vector.tensor_mul(t02[:], g[0][:], g[2][:])
                    nc.vector.tensor_mul(t01[:], g[0][:], g[1][:])
                    nc.vector.tensor_mul(mp3[:], t01[:], g[2][:])

                nv = fills[ri]
                if "scatter0" not in skip:
                    for k, src in ((0, mp3), (1, mp3), (2, mp3)):
                        nc.gpsimd.dma_scatter_add(
                            out_ap=out0r[ri % rot][:],
                            in_ap=src[:],
                            idxs_ap=sl_t[k][:, iw],
                            num_idxs=CAP,
                            num_idxs_reg=nv,
                            elem_size=E,
                            queue_num=1,
                            single_packet=single_packet,
                        )
                if "scatter1" not in skip:
                    for k, src in ((0, t12), (1, t02), (2, t01)):
                        nc.gpsimd.dma_scatter_add(
                            out_ap=out1r[ri % rot][:],
                            in_ap=src[:],
                            idxs_ap=sl_t[k][:, iw],
                            num_idxs=CAP,
                            num_idxs_reg=nv,
                            elem_size=E,
                            queue_num=2,
                            single_packet=single_packet,
                        )
            if rot > 1:
                for r in range(0, C, 128):
                    for base, extras in ((out0, out0r[1:]), (out1, out1r[1:])):
                        acc = wpool.tile([128, E], f32, name=f"acc_{base.name}_{r}",
                                         tag="acc", bufs=4)
                        ext = wpool.tile([128, E], f32, name=f"ext_{base.name}_{r}",
                                         tag="ext", bufs=4)
                        nc.sync.dma_start(out=acc[:], in_=base[r : r + 128, :])
                        for eb in extras:
                            nc.sync.dma_start(out=ext[:], in_=eb[r : r + 128, :])
                            nc.vector.tensor_add(acc[:], acc[:], ext[:])
                        nc.sync.dma_start(out=base[r : r + 128, :], in_=acc[:])
    nc.compile()
    return nc


def kernel(input_tensor, idx0, idx1, idx2):
    from concourse.bass_utils import run_bass_kernel_spmd
    import time as _time

    _timing = os.environ.get("BASS_KERNEL_TIMING")
    _t = [_time.perf_counter()]

    def _mark(label):
        if _timing:
            now = _time.perf_counter()
            print(f"[kernel] {label}: {now - _t[0]:.3f}s", file=sys.stderr)
            _t[0] = now

    input_tensor = np.asarray(input_tensor, dtype=np.float32)
    idx0 = np.asarray(idx0, dtype=np.int32)
    idx1 = np.asarray(idx1, dtype=np.int32)
    idx2 = np.asarray(idx2, dtype=np.int32)

    nr, fills, g_wrapped, s_wrapped = _build_index_tiles(idx0, idx1, idx2)
    _mark("index scheduling")
    nc = _build_nc(nr, fills)
    _mark("nc build+compile")

    # [C, R, F]: channel-major rows so one channel is contiguous per core
    x_t = np.ascontiguousarray(input_tensor.transpose(2, 1, 0))
    in_maps = []
    for m in range(NCORES):
        shard = np.ascontiguousarray(
            x_t[:, m * RS : (m + 1) * RS, :].reshape(C, E)
        )
        im = {"xt": shard}
        for k in range(3):
            im[f"gl{k}"] = g_wrapped[k]
            im[f"sl{k}"] = s_wrapped[k]
        in_maps.append(im)

    _mark("shard/transpose inputs")
    res = run_bass_kernel_spmd(nc, in_maps, core_ids=list(range(NCORES)))
    _mark("device run (incl jit+transfer)")

    part0 = np.empty((F_IN, R, C), np.float32)
    part1 = np.empty((F_IN, R, C), np.float32)
    for m in range(NCORES):
        o0 = res.results[m]["out0"].reshape(C, RS, F_IN)
        o1 = res.results[m]["out1"].reshape(C, RS, F_IN)
        part0[:, m * RS : (m + 1) * RS, :] = o0.transpose(2, 1, 0)
        part1[:, m * RS : (m + 1) * RS, :] = o1.transpose(2, 1, 0)
    out = np.concatenate([part0, part1], axis=0)
    _mark("output reassembly")
    return out
